# revision 26
# baseline (speedup 1.0000x reference)
"""Multi-head attention (B=4, L=2048, D=512, H=8) on 8 TRN2 NeuronCores.

Sharding: core c handles batch b = c//2 and head-group hg = c%2 (4 heads,
256 head-dims).  Each core computes, fully on device:
  - attn[b, hg*4:(hg+1)*4]  (normalized softmax probabilities, f32)
  - outT_part[512, 2048]    (wo-projected output^T partial over its 256
                             head-dims; host sums the 2 partials per batch,
                             transposes, and adds wo_b)

Device dataflow per core:
  proj:  qpT_h/kpT_h [65,2048] f32r = W q^T per head (bias via DVE evac;
         row 64 = ones / -1e9*mask for the score-side mask fold)
         vp [2048,256] bf16 (bias via homogeneous input row)
  A:     scores[q,k] psum = qpT.T @ kpT (K=65 folds the additive mask)
         -> ACT exp (+row sums) -> DVE 1/sum -> DVE in-place normalize
         -> DMA attn out
  B:     scores^T[k,q] psum = kpT.T @ qpT -> ACT exp with per-partition
         bias -1e9*mask[k] -> bf16 exp_b tiles
  V:     out^T[d,q] psum += vp_chunk.T @ exp_b -> DVE multiply by
         recip(rowsum) (replicated via PE transpose + DRAM-roundtrip
         broadcast) -> attn_outT (f32r)
  WO:    out_projT[512,2048] = woT.T @ attn_outT -> DMA out
"""

import numpy as np
from contextlib import ExitStack

import concourse.tile as tile
from concourse import bacc, mybir
from concourse import bass_utils
from concourse.masks import make_identity

F32 = mybir.dt.float32
F32R = mybir.dt.float32r
BF16 = mybir.dt.bfloat16
AF = mybir.ActivationFunctionType
ALU = mybir.AluOpType

B, L, D, H = 4, 2048, 512, 8
DH = 64            # head dim
NHC = 4            # heads per core
HD = NHC * DH      # 256 head-dims per core
N_CORES = 8
NEG = -1.0e9

KC = D // 128       # 4 contraction chunks for projections
QB = L // 128       # 16 q-blocks
LC = L // 512       # 4 L-chunks of 512

_NC_CACHE = {}
_last_in_maps = None


def _build_nc():
    nc = bacc.Bacc("TRN2", target_bir_lowering=False, debug=False,
                   num_devices=N_CORES)

    # ---- DRAM I/O (per-core shards; host preps layouts) ----
    qT_d = nc.dram_tensor("qT", [D, L], F32R, kind="ExternalInput").ap()
    kT_d = nc.dram_tensor("kT", [D, L], F32R, kind="ExternalInput").ap()
    vT_d = nc.dram_tensor("vT", [D + 1, L], F32R, kind="ExternalInput").ap()
    wqT_d = nc.dram_tensor("wqT", [D, HD], F32R, kind="ExternalInput").ap()
    wkT_d = nc.dram_tensor("wkT", [D, HD], F32R, kind="ExternalInput").ap()
    wvT_d = nc.dram_tensor("wvT", [D + 1, HD], F32R, kind="ExternalInput").ap()
    woT_d = nc.dram_tensor("woT", [HD, D], F32R, kind="ExternalInput").ap()
    bq_d = nc.dram_tensor("bq", [HD, 1], F32, kind="ExternalInput").ap()
    bk_d = nc.dram_tensor("bk", [HD, 1], F32, kind="ExternalInput").ap()
    maskrow_d = nc.dram_tensor("maskrow", [1, L], F32R, kind="ExternalInput").ap()
    maskT_d = nc.dram_tensor("maskT", [128, 16], F32, kind="ExternalInput").ap()

    attn_d = nc.dram_tensor("attn_out", [NHC, L, L], F32, kind="ExternalOutput").ap()
    outT_d = nc.dram_tensor("outT_part", [D, L], F32, kind="ExternalOutput").ap()

    recipT_dram = nc.dram_tensor("recipT_scratch", [64, 128], F32,
                                 kind="Internal").ap()

    with tile.TileContext(nc) as tc:
        with ExitStack() as ctx:
            # ---------- persistent pools ----------
            consts = ctx.enter_context(tc.tile_pool(name="consts", bufs=1))
            wop = ctx.enter_context(tc.tile_pool(name="wop", bufs=1))
            vppool = ctx.enter_context(tc.tile_pool(name="vppool", bufs=1))
            qkpT = ctx.enter_context(tc.tile_pool(name="qkpT", bufs=1))

            pa = ctx.enter_context(tc.tile_pool(name="pa", bufs=1, space="PSUM"))
            pb = ctx.enter_context(tc.tile_pool(name="pb", bufs=2, space="PSUM"))

            # ---------- constants ----------
            ident = consts.tile([128, 128], F32)
            make_identity(nc, ident[:])
            maskT_s = consts.tile([128, 16], F32)
            nc.sync.dma_start(maskT_s[:], maskT_d[:, :])
            bq_s = consts.tile([128, 2], F32)   # col j: heads 2j,2j+1 stacked
            bk_s = consts.tile([128, 2], F32)
            for j in range(2):
                nc.sync.dma_start(bq_s[:, j:j + 1], bq_d[j * 128:(j + 1) * 128, :])
                nc.sync.dma_start(bk_s[:, j:j + 1], bk_d[j * 128:(j + 1) * 128, :])
            recip_store = consts.tile([128, 64], F32)  # col h*16+qb
            vT5 = consts.tile([1, L], F32R)
            nc.sync.dma_start(vT5[:], vT_d[D:D + 1, :])
            wo_s = wop.tile([128, 2 * D], F32R)  # pair j rows at cols j*D
            for j in range(2):
                nc.sync.dma_start(wo_s[:, j * D:(j + 1) * D],
                                  woT_d[j * 128:(j + 1) * 128, :])

            # persistent per-head projection tiles (row 64 = ones / mask)
            qpT_t = [qkpT.tile([65, L], F32R, name=f"qpT{h}") for h in range(NHC)]
            kpT_t = [qkpT.tile([65, L], F32R, name=f"kpT{h}") for h in range(NHC)]
            vp_tiles = [vppool.tile([128, HD], BF16, name=f"vp{lt}")
                        for lt in range(QB)]

            # pools needed at the very start of attention: allocate BEFORE the
            # phase-1 pools so they land in fresh SBUF (no release-barrier dep)
            attn_p = ctx.enter_context(tc.tile_pool(name="attn", bufs=3))
            sums_p = ctx.enter_context(tc.tile_pool(name="sums", bufs=4))
            rt_p = ctx.enter_context(tc.tile_pool(name="rt", bufs=2))
            outw_p = ctx.enter_context(tc.tile_pool(name="outw", bufs=2))

            # ---------- phase 1: load inputs + all projections ----------
            with ExitStack() as ph1:
                inbufs = ph1.enter_context(tc.tile_pool(name="inbufs", bufs=1))
                wqkv = ph1.enter_context(tc.tile_pool(name="wqkv", bufs=1))

                # q/k first: the attention A/B chains only need qpT/kpT, so
                # front-load those to start A(0) as early as possible
                wq_s = wqkv.tile([128, KC * HD], F32R)
                wk_s = wqkv.tile([128, KC * HD], F32R)
                wv_s = wqkv.tile([128, KC * HD], F32R)
                for kc in range(KC):
                    nc.sync.dma_start(wq_s[:, kc * HD:(kc + 1) * HD],
                                      wqT_d[kc * 128:(kc + 1) * 128, :])
                    nc.sync.dma_start(wk_s[:, kc * HD:(kc + 1) * HD],
                                      wkT_d[kc * 128:(kc + 1) * 128, :])
                qT_tiles = []
                kT_tiles = []
                for kc in range(KC):
                    tq = inbufs.tile([128, L], F32R, tag=f"in{kc}")
                    nc.sync.dma_start(tq[:], qT_d[kc * 128:(kc + 1) * 128, :])
                    qT_tiles.append(tq)
                    tk = inbufs.tile([128, L], F32R, tag=f"ink{kc}")
                    nc.sync.dma_start(tk[:], kT_d[kc * 128:(kc + 1) * 128, :])
                    kT_tiles.append(tk)

                for h in range(NHC):
                    j, dlt = divmod(h, 2)
                    for (w_s, b_s, src, dst) in (
                            (wq_s, bq_s, qT_tiles, qpT_t[h]),
                            (wk_s, bk_s, kT_tiles, kpT_t[h])):
                        for lc in range(LC):
                            ps = pb.tile([128, 1024], F32, tag="pb")
                            psq = ps[0:64, 0:512]
                            for kc in range(KC):
                                nc.tensor.matmul(
                                    psq,
                                    w_s[:, kc * HD + h * DH:kc * HD + (h + 1) * DH],
                                    src[kc][:, lc * 512:(lc + 1) * 512],
                                    start=(kc == 0), stop=(kc == KC - 1))
                            nc.vector.tensor_scalar(
                                dst[0:64, lc * 512:(lc + 1) * 512], psq,
                                b_s[dlt * 64:(dlt + 1) * 64, j:j + 1], None,
                                ALU.add)
                    # row 64: ones for qpT (vT's homogeneous row is all ones),
                    # -1e9*mask for kpT — the A-side additive-mask fold
                    nc.sync.dma_start(qpT_t[h][64:65, :], vT_d[D:D + 1, :])
                    nc.sync.dma_start(kpT_t[h][64:65, :], maskrow_d[:, :])

                # V projection after q/k (vT reuses the qT input slots)
                for kc in range(KC):
                    nc.sync.dma_start(wv_s[:, kc * HD:(kc + 1) * HD],
                                      wvT_d[kc * 128:(kc + 1) * 128, :])
                wv5_s = wqkv.tile([1, HD], F32R)
                nc.sync.dma_start(wv5_s[:], wvT_d[D:D + 1, :])
                vT_tiles = []
                for kc in range(KC):
                    t = inbufs.tile([128, L], F32R, tag=f"in{kc}")
                    nc.sync.dma_start(t[:], vT_d[kc * 128:(kc + 1) * 128, :])
                    vT_tiles.append(t)
                for lt in range(QB):
                    ps = pb.tile([128, 1024], F32, tag="pb")
                    psv = ps[:, 0:HD]
                    for kc in range(KC):
                        nc.tensor.matmul(
                            psv, vT_tiles[kc][:, lt * 128:(lt + 1) * 128],
                            wv_s[:, kc * HD:(kc + 1) * HD],
                            start=(kc == 0), stop=False)
                    nc.tensor.matmul(psv, vT5[:, lt * 128:(lt + 1) * 128],
                                     wv5_s[:], start=False, stop=True)
                    nc.vector.tensor_copy(vp_tiles[lt][:], psv)

            # ---------- phase 2: attention ----------
            with ExitStack() as ph2:
                expb_p = ph2.enter_context(tc.tile_pool(name="expb", bufs=1))
                aoT_p = ph2.enter_context(tc.tile_pool(name="aoT", bufs=1))
                rrep_p = ph2.enter_context(tc.tile_pool(name="rrep", bufs=1))

                aoT_pair = [aoT_p.tile([128, L], F32R, name=f"aoT{j}")
                            for j in range(2)]

                def a_round(h, qb):
                    qpT, kpT = qpT_t[h], kpT_t[h]
                    attn_t = attn_p.tile([128, L], F32, tag="attn")
                    sums = sums_p.tile([128, 1], F32, tag="sums")
                    rcol = recip_store[:, h * 16 + qb:h * 16 + qb + 1]
                    ps = pa.tile([128, L], F32, tag="pa")
                    for kc in range(LC):
                        nc.tensor.matmul(
                            ps[:, kc * 512:(kc + 1) * 512],
                            qpT[:, qb * 128:(qb + 1) * 128],
                            kpT[:, kc * 512:(kc + 1) * 512],
                            start=True, stop=True)
                    nc.scalar.activation(attn_t[:], ps[:], AF.Exp,
                                         accum_out=sums[:])
                    nc.vector.reciprocal(rcol, sums[:])
                    nc.vector.tensor_scalar(
                        attn_t[:], attn_t[:], rcol, None, ALU.mult)
                    nc.gpsimd.dma_start(
                        attn_d[h, qb * 128:(qb + 1) * 128, :], attn_t[:])

                def recip_head(h, rrep):
                    """recip_store cols [h*16, h*16+16) -> rrep partition half."""
                    dlt = h % 2
                    if dlt == 0:
                        rrep = rrep_p.tile([128, L], F32, tag="rrep")
                    ps = pb.tile([128, 1024], F32, tag="pb")
                    pst = ps[0:16, 0:128]
                    nc.tensor.transpose(pst, recip_store[:, h * 16:(h + 1) * 16],
                                        ident[:])
                    rT = rt_p.tile([16, 128], F32, tag="rt")
                    nc.vector.tensor_copy(rT[:], pst)
                    nc.sync.dma_start(recipT_dram[h * 16:(h + 1) * 16, :], rT[:])
                    src = recipT_dram[h * 16:(h + 1) * 16, :]
                    src = src.rearrange("a b -> (a b)").unsqueeze(0)
                    src = src.partition_broadcast(64)
                    nc.sync.dma_start(rrep[dlt * 64:(dlt + 1) * 64, :], src)
                    return rrep

                def b_round(h, kb):
                    qpT, kpT = qpT_t[h], kpT_t[h]
                    e = expb_p.tile([128, L], BF16, tag=f"expb{kb}")
                    for half in range(2):
                        ps = pb.tile([128, 1024], F32, tag="pb")
                        for i in range(2):
                            qc = half * 2 + i
                            nc.tensor.matmul(
                                ps[:, i * 512:(i + 1) * 512],
                                kpT[0:64, kb * 128:(kb + 1) * 128],
                                qpT[0:64, qc * 512:(qc + 1) * 512],
                                start=True, stop=True)
                        nc.scalar.activation(
                            e[:, half * 1024:(half + 1) * 1024], ps[:],
                            AF.Exp, bias=maskT_s[:, kb:kb + 1])
                    return e

                def v_chunk(h, qc, exps, rrep):
                    j, dlt = divmod(h, 2)
                    ps = pb.tile([128, 1024], F32, tag="pb")
                    pv = ps[dlt * 64:(dlt + 1) * 64, 0:512]
                    for kb in range(QB):
                        nc.tensor.matmul(
                            pv, vp_tiles[kb][:, h * DH:(h + 1) * DH],
                            exps[kb][:, qc * 512:(qc + 1) * 512],
                            start=(kb == 0), stop=(kb == QB - 1),
                            tile_position=(0, dlt * 64))
                    nc.vector.tensor_tensor(
                        out=aoT_pair[j][dlt * 64:(dlt + 1) * 64,
                                        qc * 512:(qc + 1) * 512],
                        in0=pv,
                        in1=rrep[dlt * 64:(dlt + 1) * 64,
                                 qc * 512:(qc + 1) * 512],
                        op=ALU.mult)

                # Per head: A rounds carry the attn output; B rounds (lagged 4)
                # produce exp_b; V of the PREVIOUS head interleaves into rounds
                # 0..3 so PE/ACT stay busy across head boundaries.
                prev = None  # (h, exps, rrep)
                rrep = None
                for h in range(NHC):
                    # b lags a by 4 rounds so V(prev) can finish reading the
                    # expb slots first; head 0 has no prev, so no lag needed
                    lag = 4 if prev is not None else 1
                    exps = []
                    for r in range(QB):
                        a_round(h, r)
                        if r < LC and prev is not None:
                            v_chunk(prev[0], r, prev[1], prev[2])
                        if r >= lag:
                            exps.append(b_round(h, r - lag))
                    for kb in range(QB - lag, QB):
                        exps.append(b_round(h, kb))
                    rrep = recip_head(h, rrep)
                    prev = (h, exps, rrep)

                # ---------- V of last head, interleaved with the output
                # projection (wo chunk lc only needs V(3)'s qc==lc evac) ------
                def wo_group(lc):
                    for mp in range(2):  # mt pair (2*mp, 2*mp+1) in one psum
                        ps = pb.tile([128, 1024], F32, tag="pb")
                        for i in range(2):
                            mt = 2 * mp + i
                            for j in range(2):
                                nc.tensor.matmul(
                                    ps[:, i * 512:(i + 1) * 512],
                                    wo_s[:, j * D + mt * 128:
                                         j * D + (mt + 1) * 128],
                                    aoT_pair[j][:, lc * 512:(lc + 1) * 512],
                                    start=(j == 0), stop=(j == 1))
                        ot = outw_p.tile([128, 1024], F32, tag="outw")
                        nc.vector.tensor_copy(ot[:], ps[:])
                        for i in range(2):
                            mt = 2 * mp + i
                            nc.sync.dma_start(
                                outT_d[mt * 128:(mt + 1) * 128,
                                       lc * 512:(lc + 1) * 512],
                                ot[:, i * 512:(i + 1) * 512])

                for qc in range(LC):
                    v_chunk(prev[0], qc, prev[1], prev[2])
                    if qc >= 1:
                        wo_group(qc - 1)
                wo_group(LC - 1)

    nc.compile()
    return nc


def kernel(q, k, v, mask, wq_w, wq_b, wk_w, wk_b, wv_w, wv_b, wo_w, wo_b):
    q = np.asarray(q, dtype=np.float32)
    k = np.asarray(k, dtype=np.float32)
    v = np.asarray(v, dtype=np.float32)
    mask = np.asarray(mask)
    wq_w = np.asarray(wq_w, dtype=np.float32)
    wq_b = np.asarray(wq_b, dtype=np.float32)
    wk_w = np.asarray(wk_w, dtype=np.float32)
    wk_b = np.asarray(wk_b, dtype=np.float32)
    wv_w = np.asarray(wv_w, dtype=np.float32)
    wv_b = np.asarray(wv_b, dtype=np.float32)
    wo_w = np.asarray(wo_w, dtype=np.float32)
    wo_b = np.asarray(wo_b, dtype=np.float32)

    if "nc" not in _NC_CACHE:
        _NC_CACHE["nc"] = _build_nc()
    nc = _NC_CACHE["nc"]

    S = 1.0 / np.sqrt(np.float32(DH))
    ones_row = np.ones((1, L), dtype=np.float32)
    in_maps = []
    for c in range(N_CORES):
        b, hg = divmod(c, 2)
        rows = slice(hg * HD, (hg + 1) * HD)
        mb = (NEG * mask[b]).astype(np.float32)
        in_maps.append({
            "qT": np.ascontiguousarray(q[b].T),
            "kT": np.ascontiguousarray(k[b].T),
            "vT": np.ascontiguousarray(np.vstack([v[b].T, ones_row])),
            "wqT": np.ascontiguousarray((wq_w[rows, :] * S).T),
            "wkT": np.ascontiguousarray(wk_w[rows, :].T),
            "wvT": np.ascontiguousarray(
                np.vstack([wv_w[rows, :].T, wv_b[rows][None, :]])),
            "woT": np.ascontiguousarray(wo_w[:, rows].T),
            "bq": np.ascontiguousarray((wq_b[rows] * S).reshape(HD, 1)),
            "bk": np.ascontiguousarray(wk_b[rows].reshape(HD, 1)),
            "maskrow": mb.reshape(1, L),
            "maskT": np.ascontiguousarray(mb.reshape(16, 128).T),
        })

    global _last_in_maps
    _last_in_maps = in_maps
    res = bass_utils.run_bass_kernel_spmd(nc, in_maps, core_ids=list(range(N_CORES)))

    attn = np.empty((B, H, L, L), dtype=np.float32)
    out = np.empty((B, L, D), dtype=np.float32)
    for b in range(B):
        r0 = res.results[2 * b]
        r1 = res.results[2 * b + 1]
        attn[b, 0:NHC] = r0["attn_out"]
        attn[b, NHC:H] = r1["attn_out"]
        out[b] = (r0["outT_part"] + r1["outT_part"]).T + wo_b[None, :]
    return out, attn


# revision 27
# speedup vs baseline: 1.0232x; 1.0232x over previous
"""Multi-head attention (B=4, L=2048, D=512, H=8) on 8 TRN2 NeuronCores.

Sharding: core c handles batch b = c//2 and head-group hg = c%2 (4 heads,
256 head-dims).  Each core computes, fully on device:
  - attn[b, hg*4:(hg+1)*4]  (normalized softmax probabilities, f32)
  - outT_part[512, 2048]    (wo-projected output^T partial over its 256
                             head-dims; host sums the 2 partials per batch,
                             transposes, and adds wo_b)

Device dataflow per core:
  proj:  qpT_h/kpT_h [65,2048] f32r = W q^T per head (bias via DVE evac;
         row 64 = ones / -1e9*mask for the score-side mask fold)
         vp [2048,256] bf16 (bias via homogeneous input row)
  A:     scores[q,k] psum = qpT.T @ kpT (K=65 folds the additive mask)
         -> ACT exp (+row sums) -> DVE 1/sum -> DVE in-place normalize
         -> DMA attn out
  B:     scores^T[k,q] psum = kpT.T @ qpT -> ACT exp with per-partition
         bias -1e9*mask[k] -> bf16 exp_b tiles
  V:     out^T[d,q] psum += vp_chunk.T @ exp_b -> DVE multiply by
         recip(rowsum) (replicated via PE transpose + DRAM-roundtrip
         broadcast) -> attn_outT (f32r)
  WO:    out_projT[512,2048] = woT.T @ attn_outT -> DMA out
"""

import numpy as np
import ml_dtypes
from contextlib import ExitStack

import concourse.tile as tile
from concourse import bacc, mybir
from concourse import bass_utils
from concourse.masks import make_identity

F32 = mybir.dt.float32
F32R = mybir.dt.float32r
BF16 = mybir.dt.bfloat16
AF = mybir.ActivationFunctionType
ALU = mybir.AluOpType

B, L, D, H = 4, 2048, 512, 8
DH = 64            # head dim
NHC = 4            # heads per core
HD = NHC * DH      # 256 head-dims per core
N_CORES = 8
NEG = -1.0e9

KC = D // 128       # 4 contraction chunks for projections
QB = L // 128       # 16 q-blocks
LC = L // 512       # 4 L-chunks of 512

_NC_CACHE = {}
_last_in_maps = None


def _build_nc():
    nc = bacc.Bacc("TRN2", target_bir_lowering=False, debug=False,
                   num_devices=N_CORES)

    # ---- DRAM I/O (per-core shards; host preps layouts) ----
    qT_d = nc.dram_tensor("qT", [D, L], F32R, kind="ExternalInput").ap()
    kT_d = nc.dram_tensor("kT", [D, L], F32R, kind="ExternalInput").ap()
    vT_d = nc.dram_tensor("vT", [D + 1, L], BF16, kind="ExternalInput").ap()
    wqT_d = nc.dram_tensor("wqT", [D, HD], F32R, kind="ExternalInput").ap()
    wkT_d = nc.dram_tensor("wkT", [D, HD], F32R, kind="ExternalInput").ap()
    wvT_d = nc.dram_tensor("wvT", [D + 1, HD], BF16, kind="ExternalInput").ap()
    woT_d = nc.dram_tensor("woT", [HD, D], F32R, kind="ExternalInput").ap()
    bq_d = nc.dram_tensor("bq", [HD, 1], F32, kind="ExternalInput").ap()
    bk_d = nc.dram_tensor("bk", [HD, 1], F32, kind="ExternalInput").ap()
    maskrow_d = nc.dram_tensor("maskrow", [1, L], F32R, kind="ExternalInput").ap()
    ones_d = nc.dram_tensor("onesrow", [1, L], F32R, kind="ExternalInput").ap()
    maskT_d = nc.dram_tensor("maskT", [128, 16], F32, kind="ExternalInput").ap()

    attn_d = nc.dram_tensor("attn_out", [NHC, L, L], F32, kind="ExternalOutput").ap()
    outT_d = nc.dram_tensor("outT_part", [D, L], F32, kind="ExternalOutput").ap()

    recipT_dram = nc.dram_tensor("recipT_scratch", [64, 128], F32,
                                 kind="Internal").ap()

    with tile.TileContext(nc) as tc:
        with ExitStack() as ctx:
            # ---------- persistent pools ----------
            consts = ctx.enter_context(tc.tile_pool(name="consts", bufs=1))
            wop = ctx.enter_context(tc.tile_pool(name="wop", bufs=1))
            vppool = ctx.enter_context(tc.tile_pool(name="vppool", bufs=1))
            qkpT = ctx.enter_context(tc.tile_pool(name="qkpT", bufs=1))

            pa = ctx.enter_context(tc.tile_pool(name="pa", bufs=1, space="PSUM"))
            pb = ctx.enter_context(tc.tile_pool(name="pb", bufs=2, space="PSUM"))

            # ---------- constants ----------
            ident = consts.tile([128, 128], F32)
            make_identity(nc, ident[:])
            maskT_s = consts.tile([128, 16], F32)
            nc.sync.dma_start(maskT_s[:], maskT_d[:, :])
            bq_s = consts.tile([128, 2], F32)   # col j: heads 2j,2j+1 stacked
            bk_s = consts.tile([128, 2], F32)
            for j in range(2):
                nc.sync.dma_start(bq_s[:, j:j + 1], bq_d[j * 128:(j + 1) * 128, :])
                nc.sync.dma_start(bk_s[:, j:j + 1], bk_d[j * 128:(j + 1) * 128, :])
            recip_store = consts.tile([128, 64], F32)  # col h*16+qb
            vT5 = consts.tile([1, L], BF16)
            nc.sync.dma_start(vT5[:], vT_d[D:D + 1, :])
            wo_s = wop.tile([128, 2 * D], F32R)  # pair j rows at cols j*D
            for j in range(2):
                nc.sync.dma_start(wo_s[:, j * D:(j + 1) * D],
                                  woT_d[j * 128:(j + 1) * 128, :])

            # persistent per-head projection tiles (row 64 = ones / mask)
            qpT_t = [qkpT.tile([65, L], F32R, name=f"qpT{h}") for h in range(NHC)]
            kpT_t = [qkpT.tile([65, L], F32R, name=f"kpT{h}") for h in range(NHC)]
            vp_tiles = [vppool.tile([128, HD], BF16, name=f"vp{lt}")
                        for lt in range(QB)]

            # pools needed at the very start of attention: allocate BEFORE the
            # phase-1 pools so they land in fresh SBUF (no release-barrier dep)
            attn_p = ctx.enter_context(tc.tile_pool(name="attn", bufs=3))
            sums_p = ctx.enter_context(tc.tile_pool(name="sums", bufs=4))
            rt_p = ctx.enter_context(tc.tile_pool(name="rt", bufs=2))
            outw_p = ctx.enter_context(tc.tile_pool(name="outw", bufs=2))

            # ---------- phase 1: load inputs + all projections ----------
            with ExitStack() as ph1:
                inbufs = ph1.enter_context(tc.tile_pool(name="inbufs", bufs=1))
                wqkv = ph1.enter_context(tc.tile_pool(name="wqkv", bufs=1))

                # q/k first: the attention A/B chains only need qpT/kpT, so
                # front-load those to start A(0) as early as possible
                wq_s = wqkv.tile([128, KC * HD], F32R)
                wk_s = wqkv.tile([128, KC * HD], F32R)
                wv_s = wqkv.tile([128, KC * HD], BF16)
                for kc in range(KC):
                    nc.sync.dma_start(wq_s[:, kc * HD:(kc + 1) * HD],
                                      wqT_d[kc * 128:(kc + 1) * 128, :])
                    nc.sync.dma_start(wk_s[:, kc * HD:(kc + 1) * HD],
                                      wkT_d[kc * 128:(kc + 1) * 128, :])
                qT_tiles = []
                kT_tiles = []
                for kc in range(KC):
                    tq = inbufs.tile([128, L], F32R, tag=f"in{kc}")
                    nc.sync.dma_start(tq[:], qT_d[kc * 128:(kc + 1) * 128, :])
                    qT_tiles.append(tq)
                    tk = inbufs.tile([128, L], F32R, tag=f"ink{kc}")
                    nc.sync.dma_start(tk[:], kT_d[kc * 128:(kc + 1) * 128, :])
                    kT_tiles.append(tk)

                for h in range(NHC):
                    j, dlt = divmod(h, 2)
                    for (w_s, b_s, src, dst) in (
                            (wq_s, bq_s, qT_tiles, qpT_t[h]),
                            (wk_s, bk_s, kT_tiles, kpT_t[h])):
                        for lc in range(LC):
                            ps = pb.tile([128, 1024], F32, tag="pb")
                            psq = ps[0:64, 0:512]
                            for kc in range(KC):
                                nc.tensor.matmul(
                                    psq,
                                    w_s[:, kc * HD + h * DH:kc * HD + (h + 1) * DH],
                                    src[kc][:, lc * 512:(lc + 1) * 512],
                                    start=(kc == 0), stop=(kc == KC - 1))
                            nc.vector.tensor_scalar(
                                dst[0:64, lc * 512:(lc + 1) * 512], psq,
                                b_s[dlt * 64:(dlt + 1) * 64, j:j + 1], None,
                                ALU.add)
                    # row 64: ones for qpT (vT's homogeneous row is all ones),
                    # -1e9*mask for kpT — the A-side additive-mask fold
                    nc.sync.dma_start(qpT_t[h][64:65, :], ones_d[:, :])
                    nc.sync.dma_start(kpT_t[h][64:65, :], maskrow_d[:, :])

                # V projection after q/k (vT reuses the qT input slots)
                for kc in range(KC):
                    nc.sync.dma_start(wv_s[:, kc * HD:(kc + 1) * HD],
                                      wvT_d[kc * 128:(kc + 1) * 128, :])
                wv5_s = wqkv.tile([1, HD], BF16)
                nc.sync.dma_start(wv5_s[:], wvT_d[D:D + 1, :])
                vT_tiles = []
                for kc in range(KC):
                    t = inbufs.tile([128, L], BF16, tag=f"in{kc}")
                    nc.sync.dma_start(t[:], vT_d[kc * 128:(kc + 1) * 128, :])
                    vT_tiles.append(t)
                for lt in range(QB):
                    ps = pb.tile([128, 1024], F32, tag="pb")
                    psv = ps[:, 0:HD]
                    for kc in range(KC):
                        nc.tensor.matmul(
                            psv, vT_tiles[kc][:, lt * 128:(lt + 1) * 128],
                            wv_s[:, kc * HD:(kc + 1) * HD],
                            start=(kc == 0), stop=False)
                    nc.tensor.matmul(psv, vT5[:, lt * 128:(lt + 1) * 128],
                                     wv5_s[:], start=False, stop=True)
                    nc.vector.tensor_copy(vp_tiles[lt][:], psv)

            # ---------- phase 2: attention ----------
            with ExitStack() as ph2:
                expb_p = ph2.enter_context(tc.tile_pool(name="expb", bufs=1))
                aoT_p = ph2.enter_context(tc.tile_pool(name="aoT", bufs=1))
                rrep_p = ph2.enter_context(tc.tile_pool(name="rrep", bufs=1))

                aoT_pair = [aoT_p.tile([128, L], F32R, name=f"aoT{j}")
                            for j in range(2)]

                def a_round(h, qb):
                    qpT, kpT = qpT_t[h], kpT_t[h]
                    attn_t = attn_p.tile([128, L], F32, tag="attn")
                    sums = sums_p.tile([128, 1], F32, tag="sums")
                    rcol = recip_store[:, h * 16 + qb:h * 16 + qb + 1]
                    ps = pa.tile([128, L], F32, tag="pa")
                    for kc in range(LC):
                        nc.tensor.matmul(
                            ps[:, kc * 512:(kc + 1) * 512],
                            qpT[:, qb * 128:(qb + 1) * 128],
                            kpT[:, kc * 512:(kc + 1) * 512],
                            start=True, stop=True)
                    nc.scalar.activation(attn_t[:], ps[:], AF.Exp,
                                         accum_out=sums[:])
                    nc.vector.reciprocal(rcol, sums[:])
                    nc.vector.tensor_scalar(
                        attn_t[:], attn_t[:], rcol, None, ALU.mult)
                    nc.gpsimd.dma_start(
                        attn_d[h, qb * 128:(qb + 1) * 128, :], attn_t[:])

                def recip_head(h, rrep):
                    """recip_store cols [h*16, h*16+16) -> rrep partition half."""
                    dlt = h % 2
                    if dlt == 0:
                        rrep = rrep_p.tile([128, L], F32, tag="rrep")
                    ps = pb.tile([128, 1024], F32, tag="pb")
                    pst = ps[0:16, 0:128]
                    nc.tensor.transpose(pst, recip_store[:, h * 16:(h + 1) * 16],
                                        ident[:])
                    rT = rt_p.tile([16, 128], F32, tag="rt")
                    nc.vector.tensor_copy(rT[:], pst)
                    nc.sync.dma_start(recipT_dram[h * 16:(h + 1) * 16, :], rT[:])
                    src = recipT_dram[h * 16:(h + 1) * 16, :]
                    src = src.rearrange("a b -> (a b)").unsqueeze(0)
                    src = src.partition_broadcast(64)
                    nc.sync.dma_start(rrep[dlt * 64:(dlt + 1) * 64, :], src)
                    return rrep

                def b_round(h, kb):
                    qpT, kpT = qpT_t[h], kpT_t[h]
                    e = expb_p.tile([128, L], BF16, tag=f"expb{kb}")
                    for half in range(2):
                        ps = pb.tile([128, 1024], F32, tag="pb")
                        for i in range(2):
                            qc = half * 2 + i
                            nc.tensor.matmul(
                                ps[:, i * 512:(i + 1) * 512],
                                kpT[0:64, kb * 128:(kb + 1) * 128],
                                qpT[0:64, qc * 512:(qc + 1) * 512],
                                start=True, stop=True)
                        nc.scalar.activation(
                            e[:, half * 1024:(half + 1) * 1024], ps[:],
                            AF.Exp, bias=maskT_s[:, kb:kb + 1])
                    return e

                def v_chunk(h, qc, exps, rrep):
                    j, dlt = divmod(h, 2)
                    ps = pb.tile([128, 1024], F32, tag="pb")
                    pv = ps[dlt * 64:(dlt + 1) * 64, 0:512]
                    for kb in range(QB):
                        nc.tensor.matmul(
                            pv, vp_tiles[kb][:, h * DH:(h + 1) * DH],
                            exps[kb][:, qc * 512:(qc + 1) * 512],
                            start=(kb == 0), stop=(kb == QB - 1),
                            tile_position=(0, dlt * 64))
                    nc.vector.tensor_tensor(
                        out=aoT_pair[j][dlt * 64:(dlt + 1) * 64,
                                        qc * 512:(qc + 1) * 512],
                        in0=pv,
                        in1=rrep[dlt * 64:(dlt + 1) * 64,
                                 qc * 512:(qc + 1) * 512],
                        op=ALU.mult)

                # Per head: A rounds carry the attn output; B rounds (lagged 4)
                # produce exp_b; V of the PREVIOUS head interleaves into rounds
                # 0..3 so PE/ACT stay busy across head boundaries.
                prev = None  # (h, exps, rrep)
                rrep = None
                for h in range(NHC):
                    # b lags a by 4 rounds so V(prev) can finish reading the
                    # expb slots first; head 0 has no prev, so no lag needed
                    lag = 4 if prev is not None else 1
                    exps = []
                    for r in range(QB):
                        a_round(h, r)
                        if r < LC and prev is not None:
                            v_chunk(prev[0], r, prev[1], prev[2])
                        if r >= lag:
                            exps.append(b_round(h, r - lag))
                    for kb in range(QB - lag, QB):
                        exps.append(b_round(h, kb))
                    rrep = recip_head(h, rrep)
                    prev = (h, exps, rrep)

                # ---------- V of last head, interleaved with the output
                # projection (wo chunk lc only needs V(3)'s qc==lc evac) ------
                def wo_group(lc):
                    for mp in range(2):  # mt pair (2*mp, 2*mp+1) in one psum
                        ps = pb.tile([128, 1024], F32, tag="pb")
                        for i in range(2):
                            mt = 2 * mp + i
                            for j in range(2):
                                nc.tensor.matmul(
                                    ps[:, i * 512:(i + 1) * 512],
                                    wo_s[:, j * D + mt * 128:
                                         j * D + (mt + 1) * 128],
                                    aoT_pair[j][:, lc * 512:(lc + 1) * 512],
                                    start=(j == 0), stop=(j == 1))
                        ot = outw_p.tile([128, 1024], F32, tag="outw")
                        nc.vector.tensor_copy(ot[:], ps[:])
                        for i in range(2):
                            mt = 2 * mp + i
                            nc.sync.dma_start(
                                outT_d[mt * 128:(mt + 1) * 128,
                                       lc * 512:(lc + 1) * 512],
                                ot[:, i * 512:(i + 1) * 512])

                for qc in range(LC):
                    v_chunk(prev[0], qc, prev[1], prev[2])
                    if qc >= 1:
                        wo_group(qc - 1)
                wo_group(LC - 1)

    nc.compile()
    return nc


def kernel(q, k, v, mask, wq_w, wq_b, wk_w, wk_b, wv_w, wv_b, wo_w, wo_b):
    q = np.asarray(q, dtype=np.float32)
    k = np.asarray(k, dtype=np.float32)
    v = np.asarray(v, dtype=np.float32)
    mask = np.asarray(mask)
    wq_w = np.asarray(wq_w, dtype=np.float32)
    wq_b = np.asarray(wq_b, dtype=np.float32)
    wk_w = np.asarray(wk_w, dtype=np.float32)
    wk_b = np.asarray(wk_b, dtype=np.float32)
    wv_w = np.asarray(wv_w, dtype=np.float32)
    wv_b = np.asarray(wv_b, dtype=np.float32)
    wo_w = np.asarray(wo_w, dtype=np.float32)
    wo_b = np.asarray(wo_b, dtype=np.float32)

    if "nc" not in _NC_CACHE:
        _NC_CACHE["nc"] = _build_nc()
    nc = _NC_CACHE["nc"]

    S = 1.0 / np.sqrt(np.float32(DH))
    ones_row = np.ones((1, L), dtype=np.float32)
    in_maps = []
    for c in range(N_CORES):
        b, hg = divmod(c, 2)
        rows = slice(hg * HD, (hg + 1) * HD)
        mb = (NEG * mask[b]).astype(np.float32)
        in_maps.append({
            "qT": np.ascontiguousarray(q[b].T),
            "kT": np.ascontiguousarray(k[b].T),
            "vT": np.ascontiguousarray(
                np.vstack([v[b].T, ones_row])).astype(ml_dtypes.bfloat16),
            "wqT": np.ascontiguousarray((wq_w[rows, :] * S).T),
            "wkT": np.ascontiguousarray(wk_w[rows, :].T),
            "wvT": np.ascontiguousarray(np.vstack(
                [wv_w[rows, :].T, wv_b[rows][None, :]])).astype(ml_dtypes.bfloat16),
            "woT": np.ascontiguousarray(wo_w[:, rows].T),
            "bq": np.ascontiguousarray((wq_b[rows] * S).reshape(HD, 1)),
            "bk": np.ascontiguousarray(wk_b[rows].reshape(HD, 1)),
            "maskrow": mb.reshape(1, L),
            "onesrow": ones_row,
            "maskT": np.ascontiguousarray(mb.reshape(16, 128).T),
        })

    global _last_in_maps
    _last_in_maps = in_maps
    res = bass_utils.run_bass_kernel_spmd(nc, in_maps, core_ids=list(range(N_CORES)))

    attn = np.empty((B, H, L, L), dtype=np.float32)
    out = np.empty((B, L, D), dtype=np.float32)
    for b in range(B):
        r0 = res.results[2 * b]
        r1 = res.results[2 * b + 1]
        attn[b, 0:NHC] = r0["attn_out"]
        attn[b, NHC:H] = r1["attn_out"]
        out[b] = (r0["outT_part"] + r1["outT_part"]).T + wo_b[None, :]
    return out, attn


# revision 29
# speedup vs baseline: 1.0463x; 1.0226x over previous
"""Multi-head attention (B=4, L=2048, D=512, H=8) on 8 TRN2 NeuronCores.

Sharding: core c handles batch b = c//2 and head-group hg = c%2 (4 heads,
256 head-dims).  Each core computes, fully on device:
  - attn[b, hg*4:(hg+1)*4]  (normalized softmax probabilities, f32)
  - outT_part[512, 2048]    (wo-projected output^T partial over its 256
                             head-dims; host sums the 2 partials per batch,
                             transposes, and adds wo_b)

Device dataflow per core:
  proj:  qpT_h/kpT_h [65,2048] f32r = W q^T per head (bias via DVE evac;
         row 64 = ones / -1e9*mask for the score-side mask fold)
         vp [2048,256] bf16 (bias via homogeneous input row)
  A:     scores[q,k] psum = qpT.T @ kpT (K=65 folds the additive mask)
         -> ACT exp (+row sums) -> DVE 1/sum -> DVE in-place normalize
         -> DMA attn out
  B:     scores^T[k,q] psum = kpT.T @ qpT -> ACT exp with per-partition
         bias -1e9*mask[k] -> bf16 exp_b tiles
  V:     out^T[d,q] psum += vp_chunk.T @ exp_b -> DVE multiply by
         recip(rowsum) (replicated via PE transpose + DRAM-roundtrip
         broadcast) -> attn_outT (f32r)
  WO:    out_projT[512,2048] = woT.T @ attn_outT -> DMA out
"""

import numpy as np
import ml_dtypes
from contextlib import ExitStack

import concourse.tile as tile
from concourse import bacc, mybir
from concourse import bass_utils
from concourse.masks import make_identity

F32 = mybir.dt.float32
F32R = mybir.dt.float32r
BF16 = mybir.dt.bfloat16
AF = mybir.ActivationFunctionType
ALU = mybir.AluOpType

B, L, D, H = 4, 2048, 512, 8
DH = 64            # head dim
NHC = 4            # heads per core
HD = NHC * DH      # 256 head-dims per core
N_CORES = 8
NEG = -1.0e9

KC = D // 128       # 4 contraction chunks for projections
QB = L // 128       # 16 q-blocks
LC = L // 512       # 4 L-chunks of 512

_NC_CACHE = {}
_last_in_maps = None


def _build_nc():
    nc = bacc.Bacc("TRN2", target_bir_lowering=False, debug=False,
                   num_devices=N_CORES)

    # ---- DRAM I/O (per-core shards; host preps layouts) ----
    qT_d = nc.dram_tensor("qT", [D, L], F32R, kind="ExternalInput").ap()
    kT_d = nc.dram_tensor("kT", [D, L], F32R, kind="ExternalInput").ap()
    vT_d = nc.dram_tensor("vT", [D + 1, L], BF16, kind="ExternalInput").ap()
    wqT_d = nc.dram_tensor("wqT", [D, HD], F32R, kind="ExternalInput").ap()
    wkT_d = nc.dram_tensor("wkT", [D, HD], F32R, kind="ExternalInput").ap()
    wvT_d = nc.dram_tensor("wvT", [D + 1, HD], BF16, kind="ExternalInput").ap()
    woT_d = nc.dram_tensor("woT", [HD, D], F32R, kind="ExternalInput").ap()
    bq_d = nc.dram_tensor("bq", [HD, 1], F32, kind="ExternalInput").ap()
    bk_d = nc.dram_tensor("bk", [HD, 1], F32, kind="ExternalInput").ap()
    maskrow_d = nc.dram_tensor("maskrow", [1, L], F32R, kind="ExternalInput").ap()
    ones_d = nc.dram_tensor("onesrow", [1, L], F32R, kind="ExternalInput").ap()
    maskT_d = nc.dram_tensor("maskT", [128, 16], F32, kind="ExternalInput").ap()

    attn_d = nc.dram_tensor("attn_out", [NHC, L, L], F32, kind="ExternalOutput").ap()
    outT_d = nc.dram_tensor("outT_part", [D, L], F32, kind="ExternalOutput").ap()

    recipT_dram = nc.dram_tensor("recipT_scratch", [64, 128], F32,
                                 kind="Internal").ap()

    with tile.TileContext(nc) as tc:
        with ExitStack() as ctx:
            # ---------- persistent pools ----------
            consts = ctx.enter_context(tc.tile_pool(name="consts", bufs=1))
            wop = ctx.enter_context(tc.tile_pool(name="wop", bufs=1))
            vppool = ctx.enter_context(tc.tile_pool(name="vppool", bufs=1))
            qkpT = ctx.enter_context(tc.tile_pool(name="qkpT", bufs=1))

            pa = ctx.enter_context(tc.tile_pool(name="pa", bufs=1, space="PSUM"))
            pb = ctx.enter_context(tc.tile_pool(name="pb", bufs=2, space="PSUM"))

            # ---------- constants ----------
            ident = consts.tile([128, 128], F32)
            make_identity(nc, ident[:])
            maskT_s = consts.tile([128, 16], F32)
            nc.sync.dma_start(maskT_s[:], maskT_d[:, :])
            bq_s = consts.tile([128, 2], F32)   # col j: heads 2j,2j+1 stacked
            bk_s = consts.tile([128, 2], F32)
            for j in range(2):
                nc.sync.dma_start(bq_s[:, j:j + 1], bq_d[j * 128:(j + 1) * 128, :])
                nc.sync.dma_start(bk_s[:, j:j + 1], bk_d[j * 128:(j + 1) * 128, :])
            recip_store = consts.tile([128, 64], F32)  # col h*16+qb
            vT5 = consts.tile([1, L], BF16)
            nc.sync.dma_start(vT5[:], vT_d[D:D + 1, :])
            wo_s = wop.tile([128, 2 * D], F32R)  # pair j rows at cols j*D
            for j in range(2):
                nc.sync.dma_start(wo_s[:, j * D:(j + 1) * D],
                                  woT_d[j * 128:(j + 1) * 128, :])

            # persistent per-head projection tiles (row 64 = ones / mask)
            qpT_t = [qkpT.tile([65, L], F32R, name=f"qpT{h}") for h in range(NHC)]
            kpT_t = [qkpT.tile([65, L], F32R, name=f"kpT{h}") for h in range(NHC)]
            vp_tiles = [vppool.tile([128, HD], BF16, name=f"vp{lt}")
                        for lt in range(QB)]

            # pools needed at the very start of attention: allocate BEFORE the
            # phase-1 pools so they land in fresh SBUF (no release-barrier dep)
            attn_p = ctx.enter_context(tc.tile_pool(name="attn", bufs=3))
            sums_p = ctx.enter_context(tc.tile_pool(name="sums", bufs=4))
            rt_p = ctx.enter_context(tc.tile_pool(name="rt", bufs=2))
            outw_p = ctx.enter_context(tc.tile_pool(name="outw", bufs=2))

            # ---------- phase 1: load inputs + all projections ----------
            with ExitStack() as ph1:
                inbufs = ph1.enter_context(tc.tile_pool(name="inbufs", bufs=1))
                wqkv = ph1.enter_context(tc.tile_pool(name="wqkv", bufs=1))

                # q/k first: the attention A/B chains only need qpT/kpT, so
                # front-load those to start A(0) as early as possible
                wq_s = wqkv.tile([128, KC * HD], F32R)
                wk_s = wqkv.tile([128, KC * HD], F32R)
                wv_s = wqkv.tile([128, KC * HD], BF16)
                for kc in range(KC):
                    nc.sync.dma_start(wq_s[:, kc * HD:(kc + 1) * HD],
                                      wqT_d[kc * 128:(kc + 1) * 128, :])
                    nc.sync.dma_start(wk_s[:, kc * HD:(kc + 1) * HD],
                                      wkT_d[kc * 128:(kc + 1) * 128, :])
                # row 64 of every qpT/kpT tile (ones / -1e9*mask) up front:
                # these tiny DMAs must not queue behind the bulk input loads,
                # since A(0,0)'s K=65 matmul reads them
                for h in range(NHC):
                    nc.sync.dma_start(qpT_t[h][64:65, :], ones_d[:, :])
                    nc.sync.dma_start(kpT_t[h][64:65, :], maskrow_d[:, :])

                # kT full first (every A round reads all k columns), then
                # qT in L-halves: A(0, qb=0) only needs q-block 0, so the
                # critical prologue set is weights + kT + left qT half
                kT_tiles = []
                for kc in range(KC):
                    tk = inbufs.tile([128, L], F32R, tag=f"ink{kc}")
                    nc.sync.dma_start(tk[:], kT_d[kc * 128:(kc + 1) * 128, :])
                    kT_tiles.append(tk)
                qT_half = []  # [half][kc] -> [128, 1024]
                for half in range(2):
                    tiles = []
                    for kc in range(KC):
                        tq = inbufs.tile([128, L // 2], F32R,
                                         tag=f"in{'LR'[half]}{kc}")
                        nc.sync.dma_start(
                            tq[:], qT_d[kc * 128:(kc + 1) * 128,
                                        half * 1024:(half + 1) * 1024])
                        tiles.append(tq)
                    qT_half.append(tiles)

                def src_q(kc, lc):
                    return qT_half[lc // 2][kc][:, (lc % 2) * 512:
                                                (lc % 2 + 1) * 512]

                def src_k(kc, lc):
                    return kT_tiles[kc][:, lc * 512:(lc + 1) * 512]

                for h in range(NHC):
                    j, dlt = divmod(h, 2)
                    for (w_s, b_s, src, dst) in (
                            (wk_s, bk_s, src_k, kpT_t[h]),
                            (wq_s, bq_s, src_q, qpT_t[h])):
                        for lc in range(LC):
                            ps = pb.tile([128, 1024], F32, tag="pb")
                            psq = ps[0:64, 0:512]
                            for kc in range(KC):
                                nc.tensor.matmul(
                                    psq,
                                    w_s[:, kc * HD + h * DH:kc * HD + (h + 1) * DH],
                                    src(kc, lc),
                                    start=(kc == 0), stop=(kc == KC - 1))
                            nc.vector.tensor_scalar(
                                dst[0:64, lc * 512:(lc + 1) * 512], psq,
                                b_s[dlt * 64:(dlt + 1) * 64, j:j + 1], None,
                                ALU.add)

                # V projection after q/k (vT reuses the qT input slots)
                for kc in range(KC):
                    nc.sync.dma_start(wv_s[:, kc * HD:(kc + 1) * HD],
                                      wvT_d[kc * 128:(kc + 1) * 128, :])
                wv5_s = wqkv.tile([1, HD], BF16)
                nc.sync.dma_start(wv5_s[:], wvT_d[D:D + 1, :])
                vT_tiles = []
                for kc in range(KC):
                    t = inbufs.tile([128, L], BF16, tag=f"inL{kc}")
                    nc.sync.dma_start(t[:], vT_d[kc * 128:(kc + 1) * 128, :])
                    vT_tiles.append(t)
                for lt in range(QB):
                    ps = pb.tile([128, 1024], F32, tag="pb")
                    psv = ps[:, 0:HD]
                    for kc in range(KC):
                        nc.tensor.matmul(
                            psv, vT_tiles[kc][:, lt * 128:(lt + 1) * 128],
                            wv_s[:, kc * HD:(kc + 1) * HD],
                            start=(kc == 0), stop=False)
                    nc.tensor.matmul(psv, vT5[:, lt * 128:(lt + 1) * 128],
                                     wv5_s[:], start=False, stop=True)
                    nc.vector.tensor_copy(vp_tiles[lt][:], psv)

            # ---------- phase 2: attention ----------
            with ExitStack() as ph2:
                expb_p = ph2.enter_context(tc.tile_pool(name="expb", bufs=1))
                aoT_p = ph2.enter_context(tc.tile_pool(name="aoT", bufs=1))
                rrep_p = ph2.enter_context(tc.tile_pool(name="rrep", bufs=1))

                aoT_pair = [aoT_p.tile([128, L], F32R, name=f"aoT{j}")
                            for j in range(2)]

                def a_round(h, qb):
                    qpT, kpT = qpT_t[h], kpT_t[h]
                    attn_t = attn_p.tile([128, L], F32, tag="attn")
                    sums = sums_p.tile([128, 1], F32, tag="sums")
                    rcol = recip_store[:, h * 16 + qb:h * 16 + qb + 1]
                    ps = pa.tile([128, L], F32, tag="pa")
                    for kc in range(LC):
                        nc.tensor.matmul(
                            ps[:, kc * 512:(kc + 1) * 512],
                            qpT[:, qb * 128:(qb + 1) * 128],
                            kpT[:, kc * 512:(kc + 1) * 512],
                            start=True, stop=True)
                    nc.scalar.activation(attn_t[:], ps[:], AF.Exp,
                                         accum_out=sums[:])
                    nc.vector.reciprocal(rcol, sums[:])
                    nc.vector.tensor_scalar(
                        attn_t[:], attn_t[:], rcol, None, ALU.mult)
                    nc.gpsimd.dma_start(
                        attn_d[h, qb * 128:(qb + 1) * 128, :], attn_t[:])

                def recip_head(h, rrep):
                    """recip_store cols [h*16, h*16+16) -> rrep partition half."""
                    dlt = h % 2
                    if dlt == 0:
                        rrep = rrep_p.tile([128, L], F32, tag="rrep")
                    ps = pb.tile([128, 1024], F32, tag="pb")
                    pst = ps[0:16, 0:128]
                    nc.tensor.transpose(pst, recip_store[:, h * 16:(h + 1) * 16],
                                        ident[:])
                    rT = rt_p.tile([16, 128], F32, tag="rt")
                    nc.vector.tensor_copy(rT[:], pst)
                    nc.sync.dma_start(recipT_dram[h * 16:(h + 1) * 16, :], rT[:])
                    src = recipT_dram[h * 16:(h + 1) * 16, :]
                    src = src.rearrange("a b -> (a b)").unsqueeze(0)
                    src = src.partition_broadcast(64)
                    nc.sync.dma_start(rrep[dlt * 64:(dlt + 1) * 64, :], src)
                    return rrep

                def b_round(h, kb):
                    qpT, kpT = qpT_t[h], kpT_t[h]
                    e = expb_p.tile([128, L], BF16, tag=f"expb{kb}")
                    for half in range(2):
                        ps = pb.tile([128, 1024], F32, tag="pb")
                        for i in range(2):
                            qc = half * 2 + i
                            nc.tensor.matmul(
                                ps[:, i * 512:(i + 1) * 512],
                                kpT[0:64, kb * 128:(kb + 1) * 128],
                                qpT[0:64, qc * 512:(qc + 1) * 512],
                                start=True, stop=True)
                        nc.scalar.activation(
                            e[:, half * 1024:(half + 1) * 1024], ps[:],
                            AF.Exp, bias=maskT_s[:, kb:kb + 1])
                    return e

                def v_chunk(h, qc, exps, rrep):
                    j, dlt = divmod(h, 2)
                    ps = pb.tile([128, 1024], F32, tag="pb")
                    pv = ps[dlt * 64:(dlt + 1) * 64, 0:512]
                    for kb in range(QB):
                        nc.tensor.matmul(
                            pv, vp_tiles[kb][:, h * DH:(h + 1) * DH],
                            exps[kb][:, qc * 512:(qc + 1) * 512],
                            start=(kb == 0), stop=(kb == QB - 1),
                            tile_position=(0, dlt * 64))
                    nc.vector.tensor_tensor(
                        out=aoT_pair[j][dlt * 64:(dlt + 1) * 64,
                                        qc * 512:(qc + 1) * 512],
                        in0=pv,
                        in1=rrep[dlt * 64:(dlt + 1) * 64,
                                 qc * 512:(qc + 1) * 512],
                        op=ALU.mult)

                # Per head: A rounds carry the attn output; B rounds (lagged 4)
                # produce exp_b; V of the PREVIOUS head interleaves into rounds
                # 0..3 so PE/ACT stay busy across head boundaries.
                prev = None  # (h, exps, rrep)
                rrep = None
                for h in range(NHC):
                    # b lags a by 4 rounds so V(prev) can finish reading the
                    # expb slots first; head 0 has no prev, so no lag needed
                    lag = 4 if prev is not None else 1
                    exps = []
                    for r in range(QB):
                        a_round(h, r)
                        if r < LC and prev is not None:
                            v_chunk(prev[0], r, prev[1], prev[2])
                        if r >= lag:
                            exps.append(b_round(h, r - lag))
                    for kb in range(QB - lag, QB):
                        exps.append(b_round(h, kb))
                    rrep = recip_head(h, rrep)
                    prev = (h, exps, rrep)

                # ---------- V of last head, interleaved with the output
                # projection (wo chunk lc only needs V(3)'s qc==lc evac) ------
                def wo_group(lc):
                    for mp in range(2):  # mt pair (2*mp, 2*mp+1) in one psum
                        ps = pb.tile([128, 1024], F32, tag="pb")
                        for i in range(2):
                            mt = 2 * mp + i
                            for j in range(2):
                                nc.tensor.matmul(
                                    ps[:, i * 512:(i + 1) * 512],
                                    wo_s[:, j * D + mt * 128:
                                         j * D + (mt + 1) * 128],
                                    aoT_pair[j][:, lc * 512:(lc + 1) * 512],
                                    start=(j == 0), stop=(j == 1))
                        ot = outw_p.tile([128, 1024], F32, tag="outw")
                        nc.vector.tensor_copy(ot[:], ps[:])
                        for i in range(2):
                            mt = 2 * mp + i
                            nc.sync.dma_start(
                                outT_d[mt * 128:(mt + 1) * 128,
                                       lc * 512:(lc + 1) * 512],
                                ot[:, i * 512:(i + 1) * 512])

                for qc in range(LC):
                    v_chunk(prev[0], qc, prev[1], prev[2])
                    if qc >= 1:
                        wo_group(qc - 1)
                wo_group(LC - 1)

    nc.compile()
    return nc


def kernel(q, k, v, mask, wq_w, wq_b, wk_w, wk_b, wv_w, wv_b, wo_w, wo_b):
    q = np.asarray(q, dtype=np.float32)
    k = np.asarray(k, dtype=np.float32)
    v = np.asarray(v, dtype=np.float32)
    mask = np.asarray(mask)
    wq_w = np.asarray(wq_w, dtype=np.float32)
    wq_b = np.asarray(wq_b, dtype=np.float32)
    wk_w = np.asarray(wk_w, dtype=np.float32)
    wk_b = np.asarray(wk_b, dtype=np.float32)
    wv_w = np.asarray(wv_w, dtype=np.float32)
    wv_b = np.asarray(wv_b, dtype=np.float32)
    wo_w = np.asarray(wo_w, dtype=np.float32)
    wo_b = np.asarray(wo_b, dtype=np.float32)

    if "nc" not in _NC_CACHE:
        _NC_CACHE["nc"] = _build_nc()
    nc = _NC_CACHE["nc"]

    S = 1.0 / np.sqrt(np.float32(DH))
    ones_row = np.ones((1, L), dtype=np.float32)
    in_maps = []
    for c in range(N_CORES):
        b, hg = divmod(c, 2)
        rows = slice(hg * HD, (hg + 1) * HD)
        mb = (NEG * mask[b]).astype(np.float32)
        in_maps.append({
            "qT": np.ascontiguousarray(q[b].T),
            "kT": np.ascontiguousarray(k[b].T),
            "vT": np.ascontiguousarray(
                np.vstack([v[b].T, ones_row])).astype(ml_dtypes.bfloat16),
            "wqT": np.ascontiguousarray((wq_w[rows, :] * S).T),
            "wkT": np.ascontiguousarray(wk_w[rows, :].T),
            "wvT": np.ascontiguousarray(np.vstack(
                [wv_w[rows, :].T, wv_b[rows][None, :]])).astype(ml_dtypes.bfloat16),
            "woT": np.ascontiguousarray(wo_w[:, rows].T),
            "bq": np.ascontiguousarray((wq_b[rows] * S).reshape(HD, 1)),
            "bk": np.ascontiguousarray(wk_b[rows].reshape(HD, 1)),
            "maskrow": mb.reshape(1, L),
            "onesrow": ones_row,
            "maskT": np.ascontiguousarray(mb.reshape(16, 128).T),
        })

    global _last_in_maps
    _last_in_maps = in_maps
    res = bass_utils.run_bass_kernel_spmd(nc, in_maps, core_ids=list(range(N_CORES)))

    attn = np.empty((B, H, L, L), dtype=np.float32)
    out = np.empty((B, L, D), dtype=np.float32)
    for b in range(B):
        r0 = res.results[2 * b]
        r1 = res.results[2 * b + 1]
        attn[b, 0:NHC] = r0["attn_out"]
        attn[b, NHC:H] = r1["attn_out"]
        out[b] = (r0["outT_part"] + r1["outT_part"]).T + wo_b[None, :]
    return out, attn
